# revision 49
# baseline (speedup 1.0000x reference)
"""AdditiveAttention on 8 TRN2 NeuronCores — harmonic-doubling edition.

Math: out = softmax_k(mask(sum_h w_v[h] * tanh(qp[b,q,h] + kp[b,k,h]))) @ values
with qp = queries @ W_q^T, kp = keys @ W_k^T, mask from valid_lens (B,).

tanh(u) ~= sum_{l=0..3} b_l sin(2^l * w0 * u): four harmonics in a pure
power-of-two ladder, so every level comes from the previous by one doubling:
    s[l+1] = s[l]*c[l]   (stored scaled by 1/2^(l+1))
    c[l+1] = 1 - 2*4^l*s[l]^2
No odd-harmonic Chebyshev chain. sin(2^l w0 (q+k)) factorizes by angle
addition into PE matmuls contracting over (h, level, trig).

Engine split: ACT does base sin/cos (args kept in [-pi,pi] via w0 choice),
the last-level k-side "1-cos" Square, softmax exp, and output scaling; DVE
does the doubling products (tensor_tensor, 2x fp16) and the b_l*w_v scale
columns (tensor_scalar with per-partition AP scalars, 4x), split per q/k side
so each side's chain starts as soon as its base lands; Pool (gpsimd) only
issues the low-priority input DMAs and memsets (its tensor path is slow and
triggers hard utilization throttling); PE does projections, score matmuls,
transposes, AV.

Softmax runs without the per-row max: scores are bounded by C = sum|w_v|*1.05
(host-computed), exp(s - C) <= 1 never overflows, and p is stored in bf16
whose range absorbs the small values of rows far below the bound. This takes
reduce_max off the critical path entirely.

Last-level trick: the k-side cos(8 w0 x) enters only as a matmul operand, so
it is replaced by ct = 1 - cos = 2 sin^2(4 w0 x), one ACT Square straight
from level-2 sin; the dropped constant is a per-query-row score offset,
invisible to softmax. The sign folds into the host-built scale column.

Masking: keys truncated/padded to KP (multiple of 128) >= max(valid_lens); a
rank-1 matmul row adds -60000 to padded score columns so exp underflows to 0.

Sharding: core c handles batch c//2, query rows (c%2)*256..+256.
w0 and b_l are fit host-side from the actual inputs at call time.
"""

import math
from contextlib import ExitStack

import numpy as np

import concourse.bass as bass
import concourse.mybir as mybir
import concourse.tile as tile
from concourse import bacc
from concourse.bass_utils import run_bass_kernel_spmd

B, Q, K, D, H, V = 4, 512, 512, 256, 256, 256
NCORES = 8
NQ = (B * Q) // NCORES          # 256 query rows per core
NLEV = 4                        # harmonics 2^l * w0, l = 0..3
NEGM = -60000.0
FP32 = mybir.dt.float32
FP16 = mybir.dt.float16
BF16 = mybir.dt.bfloat16
AX = mybir.AxisListType
ALU = mybir.AluOpType
ACTF = mybir.ActivationFunctionType


def fit_series(qp, kp, vls):
    """Range analysis + weighted least-squares fit of tanh on the power-of-2
    harmonic ladder. qp/kp: [b][h, *]."""
    umax, xmax = 0.0, 0.0
    for b in range(B):
        kv = kp[b][:, : vls[b]]
        umax = max(umax, (qp[b].max(1) + kv.max(1)).max(),
                   -(qp[b].min(1) + kv.min(1)).min())
        xmax = max(xmax, np.abs(qp[b]).max(), np.abs(kv).max())
    P = max(2.0 * (umax + 0.15), 4.0 * xmax + 0.08)
    w0 = 2.0 * np.pi / P
    u = np.linspace(-(umax + 0.05), umax + 0.05, 4001)
    A = np.stack([np.sin((2.0 ** l) * w0 * u) for l in range(NLEV)], 1)
    wgt = np.exp(-(u ** 2) / (2 * 1.4 ** 2)) + 1e-3
    sw = np.sqrt(wgt)[:, None]
    bco, *_ = np.linalg.lstsq(A * sw, np.tanh(u) * sw[:, 0], rcond=None)
    return float(w0), bco.astype(np.float64)


def pack_layout(KP):
    """Column offsets inside the packed (128, PX) fp16 input tensor. Order
    matters: wk|kT first (k-side spine starts first), wq|qT second, v|ident
    last (needed late)."""
    NK = KP // 128
    names = ([("wk0", H), ("wk1", H), ("kT0", KP), ("kT1", KP),
              ("wq0", H), ("wq1", H), ("qT0", NQ), ("qT1", NQ)]
             + [(f"v{i}", V) for i in range(NK)] + [("ident", 128)])
    off, x = {}, 0
    for nm, w in names:
        off[nm] = x
        x += w
    cutA = off["wq0"]            # end of k-side chunk
    cutB = off["v0"]             # end of q-side chunk
    return off, x, (cutA, cutB)


def build_nc(w0, bco, KP, expC):
    NK = KP // 128
    QW = 2 * NQ                  # q-side width (2 h-chunks)
    KW = 2 * KP                  # k-side width
    CW = QW + KW
    OFF, PX, (CUTA, CUTB) = pack_layout(KP)
    HPI = math.pi / 2
    NCOL = 2 * (NLEV + 1)        # per hc: col_0..2, colS_3, colC_3

    nc = bacc.Bacc()
    pack = nc.declare_dram_parameter("pack", [128, PX], FP16, isOutput=False)
    mo = nc.declare_dram_parameter("mo", [1, KP + 128], FP16, isOutput=False)
    cols = nc.declare_dram_parameter("cols", [128, NCOL], FP32, isOutput=False)
    out_d = nc.declare_dram_parameter("out", [NQ, V], FP32, isOutput=True)

    with TileCtx(nc) as (tc, ctx):
        inp = ctx.enter_context(tc.tile_pool(name="inp", bufs=1))
        harm = ctx.enter_context(tc.tile_pool(name="harm", bufs=1))
        qbp = ctx.enter_context(tc.tile_pool(name="qb", bufs=1))
        sm = ctx.enter_context(tc.tile_pool(name="sm", bufs=1))
        ps_pr = ctx.enter_context(tc.tile_pool(name="psP", bufs=1, space="PSUM"))
        ps_sc = ctx.enter_context(tc.tile_pool(name="psS", bufs=1, space="PSUM"))
        ps_pt = ctx.enter_context(tc.tile_pool(name="psT", bufs=1, space="PSUM"))

        # ---- tiny init on Pool: warmup tiles + bias columns (before any
        # DMA issue so the PE warmup source is ready immediately)
        warm = inp.tile([1, 128], FP16, tag="warm", name="warm")
        nc.gpsimd.memset(warm, 0.25)
        hpi = inp.tile([128, 1], FP32, tag="hpi", name="hpi")
        nc.gpsimd.memset(hpi, HPI)
        nexpc = inp.tile([128, 1], FP32, tag="nexpc", name="nexpc")
        nc.gpsimd.memset(nexpc, -expC)
        wsrc = inp.tile([128, 256], FP16, tag="wsrc", name="wsrc")
        nc.gpsimd.memset(wsrc, 0.0)

        # ---- input DMAs: the three big chunks share ONE SP ring so the
        # transfers serialize in priority order (k-side, q-side, values)
        # instead of splitting DMA bandwidth; small mo/cols ride on Pool.
        big = inp.tile([128, PX], FP16, tag="big", name="big")
        CUT0 = OFF["kT1"]        # wk0|wk1|kT0 — everything dc0 needs
        nc.sync.dma_start(out=big[:, :CUT0], in_=pack[:, :CUT0])         # wk|kT0
        nc.sync.dma_start(out=big[:, CUT0:CUTA], in_=pack[:, CUT0:CUTA])  # kT1
        nc.scalar.activation(warm, warm, ACTF.Sin, scale=0.001)  # Sin table
        nc.sync.dma_start(out=big[:, CUTA:CUTB], in_=pack[:, CUTA:CUTB])  # wq|qT
        mo_sb = inp.tile([1, KP + 128], FP16, tag="mo", name="mo_sb")
        cols_sb = inp.tile([128, NCOL], FP32, tag="cols", name="cols_sb")
        nc.gpsimd.dma_start(out=mo_sb, in_=mo[:, :])
        nc.gpsimd.dma_start(out=cols_sb, in_=cols[:, :])
        nc.sync.dma_start(out=big[:, CUTB:], in_=pack[:, CUTB:])         # v|ident

        wk_sb = [big[:, OFF[f"wk{i}"]: OFF[f"wk{i}"] + H] for i in range(2)]
        kT_sb = [big[:, OFF[f"kT{i}"]: OFF[f"kT{i}"] + KP] for i in range(2)]
        wq_sb = [big[:, OFF[f"wq{i}"]: OFF[f"wq{i}"] + H] for i in range(2)]
        qT_sb = [big[:, OFF[f"qT{i}"]: OFF[f"qT{i}"] + NQ] for i in range(2)]
        v_sb = [big[:, OFF[f"v{i}"]: OFF[f"v{i}"] + V] for i in range(NK)]
        mrow_sb = mo_sb[:, :KP]
        ones_r = mo_sb[:, KP: KP + 128]
        # identity is stored with bf16 bit patterns (host side); view it as
        # bf16 so the transpose dtype matches the bf16 probabilities
        ident = big[:, OFF["ident"]: OFF["ident"] + 128].bitcast(BF16)

        def colAP(hc, j):
            return cols_sb[:, hc * (NLEV + 1) + j: hc * (NLEV + 1) + j + 1]

        # ---- PE warmup: the PE p-state ramps over ~3us of activity; dummy
        # matmuls on a memset tile bring it to full clock before real work
        wdst = ps_sc.tile([128, KP], FP32, tag="sc0", name="wdst")
        for _ in range(17):
            nc.tensor.matmul(wdst[:, :256], wsrc[:, :128], wsrc,
                             start=True, stop=True)

        # ---- projections: kp first (k spine), then qp. Both h-chunks live in
        # one PSUM tile so each base activation covers them in a single op.
        kp_ps = ps_pr.tile([128, 2, 512], FP32, tag="kp", name="kp")
        for dc in range(2):          # dc outer: dc0 matmuls start one DMA early
            for hc in range(2):
                nc.tensor.matmul(kp_ps[:, hc, :KP],
                                 wk_sb[dc][:, 128 * hc: 128 * (hc + 1)],
                                 kT_sb[dc], start=(dc == 0), stop=(dc == 1))
        qp_ps = ps_pr.tile([128, 2, NQ], FP32, tag="qp", name="qp")
        for hc in range(2):
            for dc in range(2):
                nc.tensor.matmul(qp_ps[:, hc, :],
                                 wq_sb[dc][:, 128 * hc: 128 * (hc + 1)],
                                 qT_sb[dc], start=(dc == 0), stop=(dc == 1))

        # ---- masks open the score accumulation groups early
        sc_ps = [ps_sc.tile([128, KP], FP32, tag=f"sc{qt}", name=f"sc{qt}")
                 for qt in range(2)]
        for qt in range(2):
            nc.tensor.matmul(sc_ps[qt], ones_r, mrow_sb, start=True, stop=False)

        # ---- harmonic tiles: T[l] layout [128, 2, CW], [:,0]=s, [:,1]=c;
        # columns [0:QW) = q-side, [QW:CW) = k-side
        T = [harm.tile([128, 2, CW], FP16, tag=f"T{l}", name=f"T{l}")
             for l in range(NLEV)]
        s = [T[l][:, 0] for l in range(NLEV)]
        c = [T[l][:, 1] for l in range(NLEV)]
        sqk = [harm.tile([128, KW], FP16, tag=f"sqk{l}", name=f"sqk{l}")
               for l in range(2)]
        sqq = [harm.tile([128, QW], FP16, tag=f"sqq{l}", name=f"sqq{l}")
               for l in range(3)]
        ct3k = harm.tile([128, KW], FP16, tag="ct3k", name="ct3k")

        def ks(ap):
            return ap[:, QW:]

        def qs(ap):
            return ap[:, :QW]

        # base level 0: one activation per (fn, side) — the packed PSUM
        # projection tiles let a 2-free-dim AP cover both h-chunks at once.
        # k-side first (k spine), s before c (Sqb scales gate on s0q only).
        s0k = s[0][:, QW:].rearrange("p (h k) -> p h k", h=2)
        c0k = c[0][:, QW:].rearrange("p (h k) -> p h k", h=2)
        s0q = s[0][:, :QW].rearrange("p (h q) -> p h q", h=2)
        c0q = c[0][:, :QW].rearrange("p (h q) -> p h q", h=2)
        nc.scalar.activation(s0k, kp_ps[:, :, :KP], ACTF.Sin, scale=w0)
        nc.scalar.activation(c0k, kp_ps[:, :, :KP], ACTF.Sin, scale=w0,
                             bias=hpi)
        nc.scalar.activation(s0q, qp_ps, ACTF.Sin, scale=w0)
        nc.scalar.activation(c0q, qp_ps, ACTF.Sin, scale=w0, bias=hpi)

        # scaled q-side stationaries SCb[l] = [Sqb | Cqb], [128, 2, QW]
        SCb = [qbp.tile([128, 2, QW], FP16, tag=f"SCb{l}", name=f"SCb{l}")
               for l in range(NLEV)]

        def scale_half(l, t, eng=None):
            """SCb[l][:,t] = col * T[l][:,t] on the q side. t=0: S-half
            (gates on s_l|q only), t=1: C-half. Slices are contiguous
            [128,256] so they are safe on Pool too."""
            j = l if l < 3 else 3 + t
            for hc in range(2):
                q2 = slice(hc * NQ, (hc + 1) * NQ)
                (eng or nc.vector).tensor_scalar(
                    SCb[l][:, t, q2], T[l][:, t, q2],
                    colAP(hc, j), None, ALU.mult)

        def emit_half(l, qt, t, last=False):
            """4 matmuls: trig half t of level l into sc_ps[qt]. The S-half
            (t=0) pairs with the k-side cos moving operand and vice versa."""
            for hc in range(2):
                q128 = slice(hc * NQ + qt * 128, hc * NQ + (qt + 1) * 128)
                k_sl = slice(QW + hc * KP, QW + (hc + 1) * KP)
                if t == 0:
                    mv = c[l][:, k_sl] if l < 3 else ct3k[:, hc * KP:(hc + 1) * KP]
                else:
                    mv = s[l][:, k_sl]
                fin = last and hc == 1
                nc.tensor.matmul(sc_ps[qt], SCb[l][:, t, q128], mv,
                                 start=False, stop=fin)

        def scale_full(l):
            """Both trig halves of level l in one op per hc (same column)."""
            for hc in range(2):
                q2 = slice(hc * NQ, (hc + 1) * NQ)
                nc.vector.tensor_scalar(SCb[l][:, :, q2], T[l][:, :, q2],
                                        colAP(hc, l), None, ALU.mult)

        # ---- doubling chain on DVE. Level 0 is split per side so the k
        # spine starts right after the k base; deeper levels run full-CW
        # (fewer ops, the per-op overhead dominates at these widths).
        nc.vector.tensor_mul(sqk[0], ks(s[0]), ks(s[0]))
        nc.vector.tensor_scalar(ks(c[1]), sqk[0], -2.0, 1.0, ALU.mult, ALU.add)
        nc.vector.tensor_mul(ks(s[1]), ks(s[0]), ks(c[0]))
        scale_half(0, 0)              # Sqb0: needs s0q only — S matmuls early
        emit_half(0, 0, 0)
        emit_half(0, 1, 0)
        nc.vector.tensor_mul(sqq[0], qs(s[0]), qs(s[0]))
        nc.vector.tensor_scalar(qs(c[1]), sqq[0], -2.0, 1.0, ALU.mult, ALU.add)
        nc.vector.tensor_mul(qs(s[1]), qs(s[0]), qs(c[0]))
        scale_half(0, 1)
        emit_half(0, 0, 1)
        emit_half(0, 1, 1)
        # level 1 -> 2: k-square on DVE, q-square on ACT (its window is free)
        nc.vector.tensor_mul(sqk[1], ks(s[1]), ks(s[1]))
        nc.vector.tensor_scalar(ks(c[2]), sqk[1], -8.0, 1.0, ALU.mult, ALU.add)
        nc.scalar.activation(sqq[1], qs(s[1]), ACTF.Square)
        nc.vector.tensor_scalar(qs(c[2]), sqq[1], -8.0, 1.0, ALU.mult, ALU.add)
        nc.vector.tensor_mul(s[2], s[1], c[1])
        scale_full(1)
        emit_half(1, 0, 0)
        emit_half(1, 1, 0)
        emit_half(1, 0, 1)
        emit_half(1, 1, 1)
        # level 2 -> 3. sq2q (ACT) comes BEFORE ct3k: its consumer chain
        # (c3q -> scale3C) gates the final matmuls, while ct3k only feeds
        # the S-half. k: s3k on DVE; ct3k via ACT Square from s2k.
        nc.vector.tensor_mul(ks(s[3]), ks(s[2]), ks(c[2]))
        nc.scalar.activation(sqq[2], qs(s[2]), ACTF.Square)
        nc.scalar.activation(ct3k, ks(s[2]), ACTF.Square, scale=math.sqrt(32.0))
        # switch ACT tables to the exp set — Square works in both sets, and
        # warm2 READS sqq[1] so the 1.3us load lands in ACT's idle window
        # between sq1q and sq2q (the scheduler orders by data readiness)
        warm2 = inp.tile([128, 128], FP16, tag="warm2", name="warm2")
        nc.scalar.activation(warm2, sqq[1][:, 0:128], ACTF.Exp)
        nc.vector.tensor_scalar(qs(c[3]), sqq[2], -32.0, 1.0, ALU.mult, ALU.add)
        nc.vector.tensor_mul(qs(s[3]), qs(s[2]), qs(c[2]))
        scale_full(2)
        emit_half(2, 0, 0)
        emit_half(2, 1, 0)
        emit_half(2, 0, 1)
        emit_half(2, 1, 1)
        scale_half(3, 0)
        emit_half(3, 0, 0)
        emit_half(3, 1, 0)
        scale_half(3, 1)
        emit_half(3, 0, 1, last=True)
        emit_half(3, 1, 1, last=True)

        # ---- softmax (no per-row max: constant bound expC) + AV per q-tile.
        # pt is one double-width PSUM tile; the q-tiles use disjoint halves
        # so their transposes don't serialize on each other.
        pt = ps_pt.tile([128, 2 * NK * 128], BF16, tag="pt", name="pt")
        for qt in range(2):
            scp = sc_ps[qt]
            p_sb = sm.tile([128, KP], BF16, tag=f"p{qt}", name=f"p{qt}")
            ssum = sm.tile([128, 1], FP32, tag=f"ss{qt}", name=f"ss{qt}")
            nc.scalar.activation(p_sb, scp, ACTF.Exp, bias=nexpc,
                                 accum_out=ssum)
            rs = sm.tile([128, 1], FP32, tag=f"rs{qt}", name=f"rs{qt}")
            nc.vector.reciprocal(rs, ssum)

            ptq = pt[:, qt * NK * 128: (qt + 1) * NK * 128]
            for kc in range(NK):
                nc.tensor.transpose(ptq[:, 128 * kc: 128 * (kc + 1)],
                                    p_sb[:, 128 * kc: 128 * (kc + 1)], ident)
            pts = sm.tile([128, NK * 128], BF16, tag=f"pts{qt}", name=f"pts{qt}")
            nc.vector.tensor_copy(pts, ptq)
            av = ps_pr.tile([128, V], FP32, tag=f"av{qt}", name=f"av{qt}")
            for kc in range(NK):
                nc.tensor.matmul(av, pts[:, 128 * kc: 128 * (kc + 1)], v_sb[kc],
                                 start=(kc == 0), stop=(kc == NK - 1))
            o = sm.tile([128, V], FP32, tag=f"o{qt}", name=f"o{qt}")
            nc.scalar.activation(o, av, ACTF.Copy, scale=rs)
            nc.sync.dma_start(out=out_d[128 * qt: 128 * (qt + 1), :], in_=o)

    nc.compile()
    return nc


class TileCtx:
    """TileContext + ExitStack in one `with`."""

    def __init__(self, nc):
        self.nc = nc

    def __enter__(self):
        self.ctx = ExitStack()
        self.tc = self.ctx.enter_context(tile.TileContext(self.nc))
        return self.tc, self.ctx

    def __exit__(self, *exc):
        return self.ctx.__exit__(*exc)


def prepare(inputs):
    """Host prep: shards, fit, per-core input maps."""
    queries = np.ascontiguousarray(np.asarray(inputs["queries"], np.float32))
    keys = np.ascontiguousarray(np.asarray(inputs["keys"], np.float32))
    values = np.ascontiguousarray(np.asarray(inputs["values"], np.float32))
    vls = np.asarray(inputs["valid_lens"]).astype(np.int64)
    Wq = np.asarray(inputs["W_q"], np.float32)
    Wk = np.asarray(inputs["W_k"], np.float32)
    wv = np.asarray(inputs["w_v"], np.float32)

    # device projections run on fp16-rounded inputs; match that for ranges
    q16 = queries.astype(np.float16).astype(np.float32)
    k16 = keys.astype(np.float16).astype(np.float32)
    Wq16 = Wq.astype(np.float16).astype(np.float32)
    Wk16 = Wk.astype(np.float16).astype(np.float32)
    qp = [(Wq16 @ q16[b].T).astype(np.float32) for b in range(B)]   # [h, q]
    kp = [(Wk16 @ k16[b].T).astype(np.float32) for b in range(B)]   # [h, k]
    w0, bco = fit_series(qp, kp, vls)
    KP = 128 * max(1, int(math.ceil(vls.max() / 128.0)))
    expC = float(np.abs(wv).sum() * 1.05)    # score upper bound for exp bias

    # scale columns: per hc, [col_0, col_1, col_2, colS_3, colC_3]
    NCOL = 2 * (NLEV + 1)
    cols = np.zeros((128, NCOL), np.float32)
    for hc in range(2):
        wvh = wv[128 * hc: 128 * (hc + 1)]
        base = hc * (NLEV + 1)
        for l in range(3):
            cols[:, base + l] = wvh * bco[l] * (2.0 ** l)
        cols[:, base + 3] = -wvh * bco[3] * 8.0     # Sqb_3 (pairs with ct3k)
        cols[:, base + 4] = wvh * bco[3] * 8.0      # Cqb_3 (pairs with s3k)

    OFF, PX, _cuts = pack_layout(KP)
    NK = KP // 128
    in_maps = []
    for core in range(NCORES):
        b, qlo = core // 2, (core % 2) * NQ
        n = int(vls[b])
        pk = np.zeros((128, PX), np.float16)
        qTm = queries[b, qlo: qlo + NQ].T.astype(np.float16)        # (D, NQ)
        kTm = np.zeros((D, KP), np.float16)
        kTm[:, :n] = keys[b, :n].T.astype(np.float16)
        for i in range(2):
            pk[:, OFF[f"qT{i}"]: OFF[f"qT{i}"] + NQ] = qTm[128 * i: 128 * (i + 1)]
            pk[:, OFF[f"kT{i}"]: OFF[f"kT{i}"] + KP] = kTm[128 * i: 128 * (i + 1)]
            pk[:, OFF[f"wq{i}"]: OFF[f"wq{i}"] + H] = Wq.T[128 * i: 128 * (i + 1)].astype(np.float16)
            pk[:, OFF[f"wk{i}"]: OFF[f"wk{i}"] + H] = Wk.T[128 * i: 128 * (i + 1)].astype(np.float16)
        vm = np.zeros((KP, V), np.float16)
        vm[:n] = values[b, :n].astype(np.float16)
        for i in range(NK):
            pk[:, OFF[f"v{i}"]: OFF[f"v{i}"] + V] = vm[128 * i: 128 * (i + 1)]
        # identity with bf16(1.0)=0x3F80 bit patterns, carried in the fp16 pack
        pk[:, OFF["ident"]: OFF["ident"] + 128] = \
            (np.eye(128) * 0x3F80).astype(np.uint16).view(np.float16)
        mov = np.zeros((1, KP + 128), np.float16)
        mov[0, :KP] = np.where(np.arange(KP) < n, 0.0, NEGM).astype(np.float16)
        mov[0, KP:] = 1.0
        in_maps.append({"pack": pk, "mo": mov, "cols": cols})
    return w0, bco, KP, expC, in_maps


def kernel(**inputs):
    w0, bco, KP, expC, in_maps = prepare(inputs)
    nc = build_nc(w0, bco, KP, expC)
    res = run_bass_kernel_spmd(nc, in_maps, core_ids=list(range(NCORES)))
    out = np.zeros((B, Q, V), np.float32)
    for core in range(NCORES):
        b, qlo = core // 2, (core % 2) * NQ
        out[b, qlo: qlo + NQ] = res.results[core]["out"]
    return out


# revision 57
# speedup vs baseline: 1.0030x; 1.0030x over previous
"""AdditiveAttention on 8 TRN2 NeuronCores — harmonic-doubling edition.

Math: out = softmax_k(mask(sum_h w_v[h] * tanh(qp[b,q,h] + kp[b,k,h]))) @ values
with qp = queries @ W_q^T, kp = keys @ W_k^T, mask from valid_lens (B,).

tanh(u) ~= sum_{l=0..3} b_l sin(2^l * w0 * u): four harmonics in a pure
power-of-two ladder, so every level comes from the previous by one doubling:
    s[l+1] = s[l]*c[l]   (stored scaled by 1/2^(l+1))
    c[l+1] = 1 - 2*4^l*s[l]^2
No odd-harmonic Chebyshev chain. sin(2^l w0 (q+k)) factorizes by angle
addition into PE matmuls contracting over (h, level, trig).

Engine split: ACT does base sin/cos (args kept in [-pi,pi] via w0 choice),
the last-level k-side "1-cos" Square, softmax exp, and output scaling; DVE
does the doubling products (tensor_tensor, 2x fp16) and the b_l*w_v scale
columns (tensor_scalar with per-partition AP scalars, 4x), split per q/k side
so each side's chain starts as soon as its base lands; Pool (gpsimd) only
issues the low-priority input DMAs and memsets (its tensor path is slow and
triggers hard utilization throttling); PE does projections, score matmuls,
transposes, AV.

Softmax runs without the per-row max: scores are bounded by C = sum|w_v|*1.05
(host-computed), exp(s - C) <= 1 never overflows, and p is stored in bf16
whose range absorbs the small values of rows far below the bound. This takes
reduce_max off the critical path entirely.

Last-level trick: the k-side cos(8 w0 x) enters only as a matmul operand, so
it is replaced by ct = 1 - cos = 2 sin^2(4 w0 x), one ACT Square straight
from level-2 sin; the dropped constant is a per-query-row score offset,
invisible to softmax. The sign folds into the host-built scale column.

Masking: keys truncated/padded to KP (multiple of 128) >= max(valid_lens); a
rank-1 matmul row adds -60000 to padded score columns so exp underflows to 0.

Sharding: core c handles batch c//2, query rows (c%2)*256..+256.
w0 and b_l are fit host-side from the actual inputs at call time.
"""

import math
from contextlib import ExitStack

import numpy as np

import concourse.bass as bass
import concourse.mybir as mybir
import concourse.tile as tile
from concourse import bacc
from concourse.bass_utils import run_bass_kernel_spmd

B, Q, K, D, H, V = 4, 512, 512, 256, 256, 256
NCORES = 8
NQ = (B * Q) // NCORES          # 256 query rows per core
NLEV = 4                        # harmonics 2^l * w0, l = 0..3
NEGM = -60000.0
FP32 = mybir.dt.float32
FP16 = mybir.dt.float16
BF16 = mybir.dt.bfloat16
AX = mybir.AxisListType
ALU = mybir.AluOpType
ACTF = mybir.ActivationFunctionType


def fit_series(qp, kp, vls):
    """Range analysis + weighted least-squares fit of tanh on the power-of-2
    harmonic ladder. qp/kp: [b][h, *]."""
    umax, xmax = 0.0, 0.0
    for b in range(B):
        kv = kp[b][:, : vls[b]]
        umax = max(umax, (qp[b].max(1) + kv.max(1)).max(),
                   -(qp[b].min(1) + kv.min(1)).min())
        xmax = max(xmax, np.abs(qp[b]).max(), np.abs(kv).max())
    P = max(2.0 * (umax + 0.15), 4.0 * xmax + 0.08)
    w0 = 2.0 * np.pi / P
    u = np.linspace(-(umax + 0.05), umax + 0.05, 4001)
    A = np.stack([np.sin((2.0 ** l) * w0 * u) for l in range(NLEV)], 1)
    wgt = np.exp(-(u ** 2) / (2 * 1.4 ** 2)) + 1e-3
    sw = np.sqrt(wgt)[:, None]
    bco, *_ = np.linalg.lstsq(A * sw, np.tanh(u) * sw[:, 0], rcond=None)
    return float(w0), bco.astype(np.float64)


def pack_layout(KP):
    """Column offsets inside the packed (128, PX) fp16 input tensor. Order
    matters: wk|kT first (k-side spine starts first), wq|qT second, v|ident
    last (needed late)."""
    NK = KP // 128
    names = ([("wk0", H), ("wk1", H), ("kT0", KP), ("kT1", KP),
              ("wq0", H), ("wq1", H), ("qT0", NQ), ("qT1", NQ)]
             + [(f"v{i}", V) for i in range(NK)] + [("ident", 128)])
    off, x = {}, 0
    for nm, w in names:
        off[nm] = x
        x += w
    cutA = off["wq0"]            # end of k-side chunk
    cutB = off["v0"]             # end of q-side chunk
    return off, x, (cutA, cutB)


def build_nc(w0, bco, KP, expC):
    NK = KP // 128
    QW = 2 * NQ                  # q-side width (2 h-chunks)
    KW = 2 * KP                  # k-side width
    CW = QW + KW
    OFF, PX, (CUTA, CUTB) = pack_layout(KP)
    HPI = math.pi / 2
    NCOL = 2 * (NLEV + 2)        # per hc: col_0..2, colS_3, colC_3, colX_3

    nc = bacc.Bacc()
    pack = nc.declare_dram_parameter("pack", [128, PX], FP16, isOutput=False)
    mo = nc.declare_dram_parameter("mo", [1, KP + 128], FP16, isOutput=False)
    cols = nc.declare_dram_parameter("cols", [128, NCOL], FP32, isOutput=False)
    out_d = nc.declare_dram_parameter("out", [NQ, V], FP16, isOutput=True)

    with TileCtx(nc) as (tc, ctx):
        inp = ctx.enter_context(tc.tile_pool(name="inp", bufs=1))
        harm = ctx.enter_context(tc.tile_pool(name="harm", bufs=1))
        qbp = ctx.enter_context(tc.tile_pool(name="qb", bufs=1))
        sm = ctx.enter_context(tc.tile_pool(name="sm", bufs=1))
        ps_pr = ctx.enter_context(tc.tile_pool(name="psP", bufs=1, space="PSUM"))
        ps_sc = ctx.enter_context(tc.tile_pool(name="psS", bufs=1, space="PSUM"))
        ps_pt = ctx.enter_context(tc.tile_pool(name="psT", bufs=1, space="PSUM"))

        # ---- tiny init on Pool: warmup tiles + bias columns (before any
        # DMA issue so the PE warmup source is ready immediately)
        warm = inp.tile([1, 128], FP16, tag="warm", name="warm")
        nc.gpsimd.memset(warm, 0.25)
        hpi = inp.tile([128, 1], FP32, tag="hpi", name="hpi")
        nc.gpsimd.memset(hpi, HPI)
        nexpc = inp.tile([128, 1], FP32, tag="nexpc", name="nexpc")
        nc.gpsimd.memset(nexpc, -expC)
        wsrc = inp.tile([128, 256], FP16, tag="wsrc", name="wsrc")
        nc.gpsimd.memset(wsrc, 0.0)

        # ---- input DMAs: the three big chunks share ONE SP ring so the
        # transfers serialize in priority order (k-side, q-side, values)
        # instead of splitting DMA bandwidth; small mo/cols ride on Pool.
        big = inp.tile([128, PX], FP16, tag="big", name="big")
        CUT0 = OFF["kT1"]        # wk0|wk1|kT0 — everything dc0 needs
        nc.sync.dma_start(out=big[:, :CUT0], in_=pack[:, :CUT0])         # wk|kT0
        nc.sync.dma_start(out=big[:, CUT0:CUTA], in_=pack[:, CUT0:CUTA])  # kT1
        nc.scalar.activation(warm, warm, ACTF.Sin, scale=0.001)  # Sin table
        nc.sync.dma_start(out=big[:, CUTA:CUTB], in_=pack[:, CUTA:CUTB])  # wq|qT
        mo_sb = inp.tile([1, KP + 128], FP16, tag="mo", name="mo_sb")
        cols_sb = inp.tile([128, NCOL], FP32, tag="cols", name="cols_sb")
        nc.gpsimd.dma_start(out=mo_sb, in_=mo[:, :])
        nc.gpsimd.dma_start(out=cols_sb, in_=cols[:, :])
        nc.sync.dma_start(out=big[:, CUTB:], in_=pack[:, CUTB:])         # v|ident

        wk_sb = [big[:, OFF[f"wk{i}"]: OFF[f"wk{i}"] + H] for i in range(2)]
        kT_sb = [big[:, OFF[f"kT{i}"]: OFF[f"kT{i}"] + KP] for i in range(2)]
        wq_sb = [big[:, OFF[f"wq{i}"]: OFF[f"wq{i}"] + H] for i in range(2)]
        qT_sb = [big[:, OFF[f"qT{i}"]: OFF[f"qT{i}"] + NQ] for i in range(2)]
        v_sb = [big[:, OFF[f"v{i}"]: OFF[f"v{i}"] + V] for i in range(NK)]
        mrow_sb = mo_sb[:, :KP]
        ones_r = mo_sb[:, KP: KP + 128]
        # identity is stored with bf16 bit patterns (host side); view it as
        # bf16 so the transpose dtype matches the bf16 probabilities
        ident = big[:, OFF["ident"]: OFF["ident"] + 128].bitcast(BF16)

        def colAP(hc, j):
            return cols_sb[:, hc * (NLEV + 2) + j: hc * (NLEV + 2) + j + 1]

        # ---- PE warmup: the PE p-state ramps over ~3us of activity; dummy
        # matmuls on a memset tile bring it to full clock before real work
        wdst = ps_sc.tile([128, KP], FP32, tag="sc0", name="wdst")
        for _ in range(17):
            nc.tensor.matmul(wdst[:, :256], wsrc[:, :128], wsrc,
                             start=True, stop=True)

        # ---- projections: kp first (k spine), then qp. Both h-chunks live in
        # one PSUM tile so each base activation covers them in a single op.
        kp_ps = ps_pr.tile([128, 2, 512], FP32, tag="kp", name="kp")
        for dc in range(2):          # dc outer: dc0 matmuls start one DMA early
            for hc in range(2):
                nc.tensor.matmul(kp_ps[:, hc, :KP],
                                 wk_sb[dc][:, 128 * hc: 128 * (hc + 1)],
                                 kT_sb[dc], start=(dc == 0), stop=(dc == 1))
        qp_ps = ps_pr.tile([128, 2, NQ], FP32, tag="qp", name="qp")
        for hc in range(2):
            for dc in range(2):
                nc.tensor.matmul(qp_ps[:, hc, :],
                                 wq_sb[dc][:, 128 * hc: 128 * (hc + 1)],
                                 qT_sb[dc], start=(dc == 0), stop=(dc == 1))

        # ---- masks open the score accumulation groups early
        sc_ps = [ps_sc.tile([128, KP], FP32, tag=f"sc{qt}", name=f"sc{qt}")
                 for qt in range(2)]
        for qt in range(2):
            nc.tensor.matmul(sc_ps[qt], ones_r, mrow_sb, start=True, stop=False)

        # ---- harmonic tiles: T[l] layout [128, 2, CW], [:,0]=s, [:,1]=c;
        # columns [0:QW) = q-side, [QW:CW) = k-side
        T = [harm.tile([128, 2, CW], FP16, tag=f"T{l}", name=f"T{l}")
             for l in range(NLEV)]
        s = [T[l][:, 0] for l in range(NLEV)]
        c = [T[l][:, 1] for l in range(NLEV)]
        sqk = [harm.tile([128, KW], FP16, tag=f"sqk{l}", name=f"sqk{l}")
               for l in range(2)]
        sqq = [harm.tile([128, QW], FP16, tag=f"sqq{l}", name=f"sqq{l}")
               for l in range(3)]
        ct3k = harm.tile([128, KW], FP16, tag="ct3k", name="ct3k")

        def ks(ap):
            return ap[:, QW:]

        def qs(ap):
            return ap[:, :QW]

        # base level 0: one activation per (fn, side) — the packed PSUM
        # projection tiles let a 2-free-dim AP cover both h-chunks at once.
        # k-side first (k spine), s before c (Sqb scales gate on s0q only).
        s0k = s[0][:, QW:].rearrange("p (h k) -> p h k", h=2)
        c0k = c[0][:, QW:].rearrange("p (h k) -> p h k", h=2)
        s0q = s[0][:, :QW].rearrange("p (h q) -> p h q", h=2)
        c0q = c[0][:, :QW].rearrange("p (h q) -> p h q", h=2)
        nc.scalar.activation(s0k, kp_ps[:, :, :KP], ACTF.Sin, scale=w0)
        nc.scalar.activation(c0k, kp_ps[:, :, :KP], ACTF.Sin, scale=w0,
                             bias=hpi)
        nc.scalar.activation(s0q, qp_ps, ACTF.Sin, scale=w0)
        nc.scalar.activation(c0q, qp_ps, ACTF.Sin, scale=w0, bias=hpi)

        # scaled q-side stationaries SCb[l] = [Sqb | Cqb], [128, 2, QW]
        SCb = [qbp.tile([128, 2, QW], FP16, tag=f"SCb{l}", name=f"SCb{l}")
               for l in range(NLEV)]

        def scale_half(l, t, eng=None):
            """SCb[l][:,t] = col * T[l][:,t] on the q side. t=0: S-half
            (gates on s_l|q only), t=1: C-half. Slices are contiguous
            [128,256] so they are safe on Pool too."""
            j = l if l < 3 else 3 + t
            for hc in range(2):
                q2 = slice(hc * NQ, (hc + 1) * NQ)
                (eng or nc.vector).tensor_scalar(
                    SCb[l][:, t, q2], T[l][:, t, q2],
                    colAP(hc, j), None, ALU.mult)

        def emit_half(l, qt, t, last=False):
            """4 matmuls: trig half t of level l into sc_ps[qt]. The S-half
            (t=0) pairs with the k-side cos moving operand and vice versa."""
            for hc in range(2):
                q128 = slice(hc * NQ + qt * 128, hc * NQ + (qt + 1) * 128)
                k_sl = slice(QW + hc * KP, QW + (hc + 1) * KP)
                if t == 0:
                    mv = c[l][:, k_sl] if l < 3 else ct3k[:, hc * KP:(hc + 1) * KP]
                else:
                    mv = s[l][:, k_sl]
                fin = last and hc == 1
                nc.tensor.matmul(sc_ps[qt], SCb[l][:, t, q128], mv,
                                 start=False, stop=fin)

        def scale_full(l):
            """Both trig halves of level l in one op per hc (same column)."""
            for hc in range(2):
                q2 = slice(hc * NQ, (hc + 1) * NQ)
                nc.vector.tensor_scalar(SCb[l][:, :, q2], T[l][:, :, q2],
                                        colAP(hc, l), None, ALU.mult)

        # ---- doubling chain on DVE. Level 0 is split per side so the k
        # spine starts right after the k base; deeper levels run full-CW
        # (fewer ops, the per-op overhead dominates at these widths).
        nc.vector.tensor_mul(sqk[0], ks(s[0]), ks(s[0]))
        nc.vector.tensor_scalar(ks(c[1]), sqk[0], -2.0, 1.0, ALU.mult, ALU.add)
        nc.vector.tensor_mul(ks(s[1]), ks(s[0]), ks(c[0]))
        scale_half(0, 0)              # Sqb0: needs s0q only — S matmuls early
        emit_half(0, 0, 0)
        emit_half(0, 1, 0)
        nc.vector.tensor_mul(sqq[0], qs(s[0]), qs(s[0]))
        nc.vector.tensor_scalar(qs(c[1]), sqq[0], -2.0, 1.0, ALU.mult, ALU.add)
        nc.vector.tensor_mul(qs(s[1]), qs(s[0]), qs(c[0]))
        scale_half(0, 1)
        emit_half(0, 0, 1)
        emit_half(0, 1, 1)
        # level 1 -> 2: k-square on DVE, q-square on ACT (its window is free)
        nc.vector.tensor_mul(sqk[1], ks(s[1]), ks(s[1]))
        nc.vector.tensor_scalar(ks(c[2]), sqk[1], -8.0, 1.0, ALU.mult, ALU.add)
        nc.scalar.activation(sqq[1], qs(s[1]), ACTF.Square)
        nc.vector.tensor_scalar(qs(c[2]), sqq[1], -8.0, 1.0, ALU.mult, ALU.add)
        # s2 split per side: the k half gates ct3k and s3k, run it first
        nc.vector.tensor_mul(ks(s[2]), ks(s[1]), ks(c[1]))
        nc.vector.tensor_mul(qs(s[2]), qs(s[1]), qs(c[1]))
        scale_full(1)
        emit_half(1, 0, 0)
        emit_half(1, 1, 0)
        emit_half(1, 0, 1)
        emit_half(1, 1, 1)
        # level 2 -> 3. sq2q (ACT) comes BEFORE ct3k: its consumer chain
        # (c3q -> scale3C) gates the final matmuls, while ct3k only feeds
        # the S-half. k: s3k on DVE; ct3k via ACT Square from s2k.
        nc.vector.tensor_mul(ks(s[3]), ks(s[2]), ks(c[2]))
        nc.scalar.activation(sqq[2], qs(s[2]), ACTF.Square)
        nc.scalar.activation(ct3k, ks(s[2]), ACTF.Square, scale=math.sqrt(32.0))
        # switch ACT tables to the exp set — Square works in both sets, and
        # warm2 READS sqq[1] so the 1.3us load lands in ACT's idle window
        # between sq1q and sq2q (the scheduler orders by data readiness)
        warm2 = inp.tile([128, 128], FP16, tag="warm2", name="warm2")
        nc.scalar.activation(warm2, sqq[1][:, 0:128], ACTF.Exp)
        nc.vector.tensor_mul(qs(s[3]), qs(s[2]), qs(c[2]))
        scale_full(2)
        emit_half(2, 0, 0)
        emit_half(2, 1, 0)
        emit_half(2, 0, 1)
        emit_half(2, 1, 1)
        scale_half(3, 0)
        emit_half(3, 0, 0)
        emit_half(3, 1, 0)
        # Cqb_3 directly from sq2q: col*(1 - 32*sq) in one tensor_scalar per
        # hc — skips materializing c3q and removes a dependency hop on the
        # path that gates the final matmuls
        for hc in range(2):
            q2 = slice(hc * NQ, (hc + 1) * NQ)
            nc.vector.tensor_scalar(SCb[3][:, 1, q2], sqq[2][:, q2],
                                    colAP(hc, 5), colAP(hc, 4),
                                    ALU.mult, ALU.add)
        emit_half(3, 0, 1, last=True)
        emit_half(3, 1, 1, last=True)

        # ---- softmax (no per-row max: constant bound expC) + AV per q-tile.
        # pt is one double-width PSUM tile; the q-tiles use disjoint halves
        # so their transposes don't serialize on each other.
        pt = ps_pt.tile([128, 2 * NK * 128], BF16, tag="pt", name="pt")
        for qt in range(2):
            scp = sc_ps[qt]
            p_sb = sm.tile([128, KP], BF16, tag=f"p{qt}", name=f"p{qt}")
            ssum = sm.tile([128, 1], FP32, tag=f"ss{qt}", name=f"ss{qt}")
            nc.scalar.activation(p_sb, scp, ACTF.Exp, bias=nexpc,
                                 accum_out=ssum)
            rs = sm.tile([128, 1], FP32, tag=f"rs{qt}", name=f"rs{qt}")
            nc.vector.reciprocal(rs, ssum)

            ptq = pt[:, qt * NK * 128: (qt + 1) * NK * 128]
            for kc in range(NK):
                nc.tensor.transpose(ptq[:, 128 * kc: 128 * (kc + 1)],
                                    p_sb[:, 128 * kc: 128 * (kc + 1)], ident)
            pts = sm.tile([128, NK * 128], BF16, tag=f"pts{qt}", name=f"pts{qt}")
            nc.vector.tensor_copy(pts, ptq)
            av = ps_pr.tile([128, V], FP32, tag=f"av{qt}", name=f"av{qt}")
            for kc in range(NK):
                nc.tensor.matmul(av, pts[:, 128 * kc: 128 * (kc + 1)], v_sb[kc],
                                 start=(kc == 0), stop=(kc == NK - 1))
            o = sm.tile([128, V], FP16, tag=f"o{qt}", name=f"o{qt}")
            nc.scalar.activation(o, av, ACTF.Copy, scale=rs)
            nc.sync.dma_start(out=out_d[128 * qt: 128 * (qt + 1), :], in_=o)

    nc.compile()
    return nc


class TileCtx:
    """TileContext + ExitStack in one `with`."""

    def __init__(self, nc):
        self.nc = nc

    def __enter__(self):
        self.ctx = ExitStack()
        self.tc = self.ctx.enter_context(tile.TileContext(self.nc))
        return self.tc, self.ctx

    def __exit__(self, *exc):
        return self.ctx.__exit__(*exc)


def prepare(inputs):
    """Host prep: shards, fit, per-core input maps."""
    queries = np.ascontiguousarray(np.asarray(inputs["queries"], np.float32))
    keys = np.ascontiguousarray(np.asarray(inputs["keys"], np.float32))
    values = np.ascontiguousarray(np.asarray(inputs["values"], np.float32))
    vls = np.asarray(inputs["valid_lens"]).astype(np.int64)
    Wq = np.asarray(inputs["W_q"], np.float32)
    Wk = np.asarray(inputs["W_k"], np.float32)
    wv = np.asarray(inputs["w_v"], np.float32)

    # device projections run on fp16-rounded inputs; match that for ranges
    q16 = queries.astype(np.float16).astype(np.float32)
    k16 = keys.astype(np.float16).astype(np.float32)
    Wq16 = Wq.astype(np.float16).astype(np.float32)
    Wk16 = Wk.astype(np.float16).astype(np.float32)
    qp = [(Wq16 @ q16[b].T).astype(np.float32) for b in range(B)]   # [h, q]
    kp = [(Wk16 @ k16[b].T).astype(np.float32) for b in range(B)]   # [h, k]
    w0, bco = fit_series(qp, kp, vls)
    KP = 128 * max(1, int(math.ceil(vls.max() / 128.0)))
    expC = float(np.abs(wv).sum() * 1.05)    # score upper bound for exp bias

    # scale columns: per hc, [col_0, col_1, col_2, colS_3, colC_3, colX_3]
    NCOL = 2 * (NLEV + 2)
    cols = np.zeros((128, NCOL), np.float32)
    for hc in range(2):
        wvh = wv[128 * hc: 128 * (hc + 1)]
        base = hc * (NLEV + 2)
        for l in range(3):
            cols[:, base + l] = wvh * bco[l] * (2.0 ** l)
        cols[:, base + 3] = -wvh * bco[3] * 8.0     # Sqb_3 (pairs with ct3k)
        cols[:, base + 4] = wvh * bco[3] * 8.0      # Cqb_3 bias term
        cols[:, base + 5] = -32.0 * wvh * bco[3] * 8.0  # Cqb_3 * sq2q term

    OFF, PX, _cuts = pack_layout(KP)
    NK = KP // 128
    in_maps = []
    for core in range(NCORES):
        b, qlo = core // 2, (core % 2) * NQ
        n = int(vls[b])
        pk = np.zeros((128, PX), np.float16)
        qTm = queries[b, qlo: qlo + NQ].T.astype(np.float16)        # (D, NQ)
        kTm = np.zeros((D, KP), np.float16)
        kTm[:, :n] = keys[b, :n].T.astype(np.float16)
        for i in range(2):
            pk[:, OFF[f"qT{i}"]: OFF[f"qT{i}"] + NQ] = qTm[128 * i: 128 * (i + 1)]
            pk[:, OFF[f"kT{i}"]: OFF[f"kT{i}"] + KP] = kTm[128 * i: 128 * (i + 1)]
            pk[:, OFF[f"wq{i}"]: OFF[f"wq{i}"] + H] = Wq.T[128 * i: 128 * (i + 1)].astype(np.float16)
            pk[:, OFF[f"wk{i}"]: OFF[f"wk{i}"] + H] = Wk.T[128 * i: 128 * (i + 1)].astype(np.float16)
        vm = np.zeros((KP, V), np.float16)
        vm[:n] = values[b, :n].astype(np.float16)
        for i in range(NK):
            pk[:, OFF[f"v{i}"]: OFF[f"v{i}"] + V] = vm[128 * i: 128 * (i + 1)]
        # identity with bf16(1.0)=0x3F80 bit patterns, carried in the fp16 pack
        pk[:, OFF["ident"]: OFF["ident"] + 128] = \
            (np.eye(128) * 0x3F80).astype(np.uint16).view(np.float16)
        mov = np.zeros((1, KP + 128), np.float16)
        mov[0, :KP] = np.where(np.arange(KP) < n, 0.0, NEGM).astype(np.float16)
        mov[0, KP:] = 1.0
        in_maps.append({"pack": pk, "mo": mov, "cols": cols})
    return w0, bco, KP, expC, in_maps


def kernel(**inputs):
    w0, bco, KP, expC, in_maps = prepare(inputs)
    nc = build_nc(w0, bco, KP, expC)
    res = run_bass_kernel_spmd(nc, in_maps, core_ids=list(range(NCORES)))
    out = np.zeros((B, Q, V), np.float32)
    for core in range(NCORES):
        b, qlo = core // 2, (core % 2) * NQ
        out[b, qlo: qlo + NQ] = np.asarray(res.results[core]["out"],
                                           np.float32)
    return out


# revision 60
# speedup vs baseline: 1.1106x; 1.1073x over previous
"""AdditiveAttention on 8 TRN2 NeuronCores — harmonic-doubling edition.

Math: out = softmax_k(mask(sum_h w_v[h] * tanh(qp[b,q,h] + kp[b,k,h]))) @ values
with qp = queries @ W_q^T, kp = keys @ W_k^T, mask from valid_lens (B,).

tanh(u) ~= sum_{l=0..3} b_l sin(2^l * w0 * u): four harmonics in a pure
power-of-two ladder, so every level comes from the previous by one doubling:
    s[l+1] = s[l]*c[l]   (stored scaled by 1/2^(l+1))
    c[l+1] = 1 - 2*4^l*s[l]^2
No odd-harmonic Chebyshev chain. sin(2^l w0 (q+k)) factorizes by angle
addition into PE matmuls contracting over (h, level, trig).

Engine split: ACT does base sin/cos (args kept in [-pi,pi] via w0 choice),
the last-level k-side "1-cos" Square, softmax exp, and output scaling; DVE
does the doubling products (tensor_tensor, 2x fp16) and the b_l*w_v scale
columns (tensor_scalar with per-partition AP scalars, 4x), split per q/k side
so each side's chain starts as soon as its base lands; Pool (gpsimd) only
issues the low-priority input DMAs and memsets (its tensor path is slow and
triggers hard utilization throttling); PE does projections, score matmuls,
transposes, AV.

Softmax runs without the per-row max: scores are bounded by C = sum|w_v|*1.05
(host-computed), exp(s - C) <= 1 never overflows, and p is stored in bf16
whose range absorbs the small values of rows far below the bound. This takes
reduce_max off the critical path entirely.

Last-level trick: the k-side cos(8 w0 x) enters only as a matmul operand, so
it is replaced by ct = 1 - cos = 2 sin^2(4 w0 x), one ACT Square straight
from level-2 sin; the dropped constant is a per-query-row score offset,
invisible to softmax. The sign folds into the host-built scale column.

Masking: keys truncated/padded to KP (multiple of 128) >= max(valid_lens); a
rank-1 matmul row adds -60000 to padded score columns so exp underflows to 0.

Sharding: core c handles batch c//2, query rows (c%2)*256..+256.
w0 and b_l are fit host-side from the actual inputs at call time.
"""

import math
from contextlib import ExitStack

import numpy as np

import concourse.bass as bass
import concourse.mybir as mybir
import concourse.tile as tile
from concourse import bacc
from concourse.bass_utils import run_bass_kernel_spmd

B, Q, K, D, H, V = 4, 512, 512, 256, 256, 256
NCORES = 8
NQ = (B * Q) // NCORES          # 256 query rows per core
NLEV = 4                        # harmonics 2^l * w0, l = 0..3
NEGM = -60000.0
FP32 = mybir.dt.float32
FP16 = mybir.dt.float16
BF16 = mybir.dt.bfloat16
AX = mybir.AxisListType
ALU = mybir.AluOpType
ACTF = mybir.ActivationFunctionType


def fit_series(qp, kp, vls):
    """Range analysis + weighted least-squares fit of tanh on the power-of-2
    harmonic ladder. qp/kp: [b][h, *]."""
    umax, xmax = 0.0, 0.0
    for b in range(B):
        kv = kp[b][:, : vls[b]]
        umax = max(umax, (qp[b].max(1) + kv.max(1)).max(),
                   -(qp[b].min(1) + kv.min(1)).min())
        xmax = max(xmax, np.abs(qp[b]).max(), np.abs(kv).max())
    P = max(2.0 * (umax + 0.15), 4.0 * xmax + 0.08)
    w0 = 2.0 * np.pi / P
    u = np.linspace(-(umax + 0.05), umax + 0.05, 4001)
    A = np.stack([np.sin((2.0 ** l) * w0 * u) for l in range(NLEV)], 1)
    wgt = np.exp(-(u ** 2) / (2 * 1.4 ** 2)) + 1e-3
    sw = np.sqrt(wgt)[:, None]
    bco, *_ = np.linalg.lstsq(A * sw, np.tanh(u) * sw[:, 0], rcond=None)
    return float(w0), bco.astype(np.float64)


def pack_layout(KP):
    """Column offsets inside the packed (128, PX) fp16 input tensor. Order
    matters: wk|kT first (k-side spine starts first), wq|qT second, v|ident
    last (needed late)."""
    NK = KP // 128
    names = ([("wk0", H), ("wk1", H), ("kT0", KP), ("kT1", KP),
              ("wq0", H), ("wq1", H), ("qT0", NQ), ("qT1", NQ)]
             + [(f"v{i}", V) for i in range(NK)] + [("ident", 128)])
    off, x = {}, 0
    for nm, w in names:
        off[nm] = x
        x += w
    cutA = off["wq0"]            # end of k-side chunk
    cutB = off["v0"]             # end of q-side chunk
    return off, x, (cutA, cutB)


def build_nc(w0, bco, KP, expC):
    NK = KP // 128
    QW = 2 * NQ                  # q-side width (2 h-chunks)
    KW = 2 * KP                  # k-side width
    CW = QW + KW
    OFF, PX, (CUTA, CUTB) = pack_layout(KP)
    HPI = math.pi / 2
    NCOL = 2 * (NLEV + 2)        # per hc: col_0..2, colS_3, colC_3, colX_3

    nc = bacc.Bacc()
    pack = nc.declare_dram_parameter("pack", [128, PX], FP16, isOutput=False)
    mo = nc.declare_dram_parameter("mo", [1, KP + 128], FP16, isOutput=False)
    cols = nc.declare_dram_parameter("cols", [128, NCOL], FP32, isOutput=False)
    out_d = nc.declare_dram_parameter("out", [NQ, V], FP16, isOutput=True)

    with TileCtx(nc) as (tc, ctx):
        inp = ctx.enter_context(tc.tile_pool(name="inp", bufs=1))
        harm = ctx.enter_context(tc.tile_pool(name="harm", bufs=1))
        qbp = ctx.enter_context(tc.tile_pool(name="qb", bufs=1))
        sm = ctx.enter_context(tc.tile_pool(name="sm", bufs=1))
        ps_pr = ctx.enter_context(tc.tile_pool(name="psP", bufs=1, space="PSUM"))
        ps_sc = ctx.enter_context(tc.tile_pool(name="psS", bufs=1, space="PSUM"))
        ps_pt = ctx.enter_context(tc.tile_pool(name="psT", bufs=1, space="PSUM"))

        # ---- tiny init on Pool: warmup tiles + bias columns (before any
        # DMA issue so the PE warmup source is ready immediately)
        warm = inp.tile([1, 128], FP16, tag="warm", name="warm")
        nc.gpsimd.memset(warm, 0.25)
        hpi = inp.tile([128, 1], FP32, tag="hpi", name="hpi")
        nc.gpsimd.memset(hpi, HPI)
        nexpc = inp.tile([128, 1], FP32, tag="nexpc", name="nexpc")
        nc.gpsimd.memset(nexpc, -expC)
        wsrc = inp.tile([128, 256], FP16, tag="wsrc", name="wsrc")
        nc.gpsimd.memset(wsrc, 0.0)

        # ---- input DMAs: the three big chunks share ONE SP ring so the
        # transfers serialize in priority order (k-side, q-side, values)
        # instead of splitting DMA bandwidth; small mo/cols ride on Pool.
        big = inp.tile([128, PX], FP16, tag="big", name="big")
        CUT0 = OFF["kT1"]        # wk0|wk1|kT0 — everything dc0 needs
        nc.sync.dma_start(out=big[:, :CUT0], in_=pack[:, :CUT0])         # wk|kT0
        nc.sync.dma_start(out=big[:, CUT0:CUTA], in_=pack[:, CUT0:CUTA])  # kT1
        nc.scalar.activation(warm, warm, ACTF.Sin, scale=0.001)  # Sin table
        nc.sync.dma_start(out=big[:, CUTA:CUTB], in_=pack[:, CUTA:CUTB])  # wq|qT
        mo_sb = inp.tile([1, KP + 128], FP16, tag="mo", name="mo_sb")
        cols_sb = inp.tile([128, NCOL], FP32, tag="cols", name="cols_sb")
        nc.gpsimd.dma_start(out=mo_sb, in_=mo[:, :])
        nc.gpsimd.dma_start(out=cols_sb, in_=cols[:, :])
        nc.sync.dma_start(out=big[:, CUTB:], in_=pack[:, CUTB:])         # v|ident

        wk_sb = [big[:, OFF[f"wk{i}"]: OFF[f"wk{i}"] + H] for i in range(2)]
        kT_sb = [big[:, OFF[f"kT{i}"]: OFF[f"kT{i}"] + KP] for i in range(2)]
        wq_sb = [big[:, OFF[f"wq{i}"]: OFF[f"wq{i}"] + H] for i in range(2)]
        qT_sb = [big[:, OFF[f"qT{i}"]: OFF[f"qT{i}"] + NQ] for i in range(2)]
        v_sb = [big[:, OFF[f"v{i}"]: OFF[f"v{i}"] + V] for i in range(NK)]
        mrow_sb = mo_sb[:, :KP]
        ones_r = mo_sb[:, KP: KP + 128]
        # identity is stored with bf16 bit patterns (host side); view it as
        # bf16 so the transpose dtype matches the bf16 probabilities
        ident = big[:, OFF["ident"]: OFF["ident"] + 128].bitcast(BF16)

        def colAP(hc, j):
            return cols_sb[:, hc * (NLEV + 2) + j: hc * (NLEV + 2) + j + 1]

        # ---- PE warmup: the PE p-state ramps over ~3us of activity; dummy
        # matmuls on a memset tile bring it to full clock before real work
        wdst = ps_sc.tile([128, KP], FP32, tag="sc0", name="wdst")
        for _ in range(17):
            nc.tensor.matmul(wdst[:, :256], wsrc[:, :128], wsrc,
                             start=True, stop=True)

        # ---- projections: kp first (k spine), then qp. Both h-chunks live in
        # one PSUM tile so each base activation covers them in a single op.
        kp_ps = ps_pr.tile([128, 2, 512], FP32, tag="kp", name="kp")
        for dc in range(2):          # dc outer: dc0 matmuls start one DMA early
            for hc in range(2):
                nc.tensor.matmul(kp_ps[:, hc, :KP],
                                 wk_sb[dc][:, 128 * hc: 128 * (hc + 1)],
                                 kT_sb[dc], start=(dc == 0), stop=(dc == 1))
        qp_ps = ps_pr.tile([128, 2, NQ], FP32, tag="qp", name="qp")
        for hc in range(2):
            for dc in range(2):
                nc.tensor.matmul(qp_ps[:, hc, :],
                                 wq_sb[dc][:, 128 * hc: 128 * (hc + 1)],
                                 qT_sb[dc], start=(dc == 0), stop=(dc == 1))

        # ---- masks open the score accumulation groups early
        sc_ps = [ps_sc.tile([128, KP], FP32, tag=f"sc{qt}", name=f"sc{qt}")
                 for qt in range(2)]
        for qt in range(2):
            nc.tensor.matmul(sc_ps[qt], ones_r, mrow_sb, start=True, stop=False)

        # PE filler matmuls: the PE clock drops to ~half after any >0.5us
        # idle gap and stays there, so dummy matmuls (into the pt bank,
        # which is unused until the AV stage) bridge the known stalls while
        # DVE produces the first scale columns.
        pt_tile = ps_pt.tile([128, 2 * NK * 128], BF16, tag="pt", name="pt")
        ptf = pt_tile[:, :512].bitcast(FP32)

        def fillers(n):
            for _ in range(n):
                nc.tensor.matmul(ptf[:, :256], wsrc[:, :128], wsrc,
                                 start=True, stop=True)

        fillers(6)

        # ---- harmonic tiles: T[l] layout [128, 2, CW], [:,0]=s, [:,1]=c;
        # columns [0:QW) = q-side, [QW:CW) = k-side
        T = [harm.tile([128, 2, CW], FP16, tag=f"T{l}", name=f"T{l}")
             for l in range(NLEV)]
        s = [T[l][:, 0] for l in range(NLEV)]
        c = [T[l][:, 1] for l in range(NLEV)]
        sqk = [harm.tile([128, KW], FP16, tag=f"sqk{l}", name=f"sqk{l}")
               for l in range(2)]
        sqq = [harm.tile([128, QW], FP16, tag=f"sqq{l}", name=f"sqq{l}")
               for l in range(3)]
        ct3k = harm.tile([128, KW], FP16, tag="ct3k", name="ct3k")

        def ks(ap):
            return ap[:, QW:]

        def qs(ap):
            return ap[:, :QW]

        # base level 0: one activation per (fn, side) — the packed PSUM
        # projection tiles let a 2-free-dim AP cover both h-chunks at once.
        # k-side first (k spine), s before c (Sqb scales gate on s0q only).
        s0k = s[0][:, QW:].rearrange("p (h k) -> p h k", h=2)
        c0k = c[0][:, QW:].rearrange("p (h k) -> p h k", h=2)
        s0q = s[0][:, :QW].rearrange("p (h q) -> p h q", h=2)
        c0q = c[0][:, :QW].rearrange("p (h q) -> p h q", h=2)
        nc.scalar.activation(s0k, kp_ps[:, :, :KP], ACTF.Sin, scale=w0)
        nc.scalar.activation(c0k, kp_ps[:, :, :KP], ACTF.Sin, scale=w0,
                             bias=hpi)
        nc.scalar.activation(s0q, qp_ps, ACTF.Sin, scale=w0)
        nc.scalar.activation(c0q, qp_ps, ACTF.Sin, scale=w0, bias=hpi)

        # scaled q-side stationaries SCb[l] = [Sqb | Cqb], [128, 2, QW]
        SCb = [qbp.tile([128, 2, QW], FP16, tag=f"SCb{l}", name=f"SCb{l}")
               for l in range(NLEV)]

        def scale_half(l, t, eng=None):
            """SCb[l][:,t] = col * T[l][:,t] on the q side. t=0: S-half
            (gates on s_l|q only), t=1: C-half. Slices are contiguous
            [128,256] so they are safe on Pool too."""
            j = l if l < 3 else 3 + t
            for hc in range(2):
                q2 = slice(hc * NQ, (hc + 1) * NQ)
                (eng or nc.vector).tensor_scalar(
                    SCb[l][:, t, q2], T[l][:, t, q2],
                    colAP(hc, j), None, ALU.mult)

        def emit_half(l, qt, t, last=False):
            """4 matmuls: trig half t of level l into sc_ps[qt]. The S-half
            (t=0) pairs with the k-side cos moving operand and vice versa."""
            for hc in range(2):
                q128 = slice(hc * NQ + qt * 128, hc * NQ + (qt + 1) * 128)
                k_sl = slice(QW + hc * KP, QW + (hc + 1) * KP)
                if t == 0:
                    mv = c[l][:, k_sl] if l < 3 else ct3k[:, hc * KP:(hc + 1) * KP]
                else:
                    mv = s[l][:, k_sl]
                fin = last and hc == 1
                nc.tensor.matmul(sc_ps[qt], SCb[l][:, t, q128], mv,
                                 start=False, stop=fin)

        def scale_full(l):
            """Both trig halves of level l in one op per hc (same column)."""
            for hc in range(2):
                q2 = slice(hc * NQ, (hc + 1) * NQ)
                nc.vector.tensor_scalar(SCb[l][:, :, q2], T[l][:, :, q2],
                                        colAP(hc, l), None, ALU.mult)

        # ---- doubling chain on DVE. Level 0 is split per side so the k
        # spine starts right after the k base; deeper levels run full-CW
        # (fewer ops, the per-op overhead dominates at these widths).
        nc.vector.tensor_mul(sqk[0], ks(s[0]), ks(s[0]))
        nc.vector.tensor_scalar(ks(c[1]), sqk[0], -2.0, 1.0, ALU.mult, ALU.add)
        nc.vector.tensor_mul(ks(s[1]), ks(s[0]), ks(c[0]))
        scale_half(0, 0)              # Sqb0: needs s0q only — S matmuls early
        emit_half(0, 0, 0)
        emit_half(0, 1, 0)
        nc.vector.tensor_mul(sqq[0], qs(s[0]), qs(s[0]))
        nc.vector.tensor_scalar(qs(c[1]), sqq[0], -2.0, 1.0, ALU.mult, ALU.add)
        nc.vector.tensor_mul(qs(s[1]), qs(s[0]), qs(c[0]))
        scale_half(0, 1)
        emit_half(0, 0, 1)
        emit_half(0, 1, 1)
        fillers(4)
        # level 1 -> 2: k-square on DVE, q-square on ACT (its window is free)
        nc.vector.tensor_mul(sqk[1], ks(s[1]), ks(s[1]))
        nc.vector.tensor_scalar(ks(c[2]), sqk[1], -8.0, 1.0, ALU.mult, ALU.add)
        nc.scalar.activation(sqq[1], qs(s[1]), ACTF.Square)
        nc.vector.tensor_scalar(qs(c[2]), sqq[1], -8.0, 1.0, ALU.mult, ALU.add)
        # s2 split per side: the k half gates ct3k and s3k, run it first
        nc.vector.tensor_mul(ks(s[2]), ks(s[1]), ks(c[1]))
        nc.vector.tensor_mul(qs(s[2]), qs(s[1]), qs(c[1]))
        scale_full(1)
        emit_half(1, 0, 0)
        emit_half(1, 1, 0)
        emit_half(1, 0, 1)
        emit_half(1, 1, 1)
        # level 2 -> 3. sq2q (ACT) comes BEFORE ct3k: its consumer chain
        # (c3q -> scale3C) gates the final matmuls, while ct3k only feeds
        # the S-half. k: s3k on DVE; ct3k via ACT Square from s2k.
        nc.vector.tensor_mul(ks(s[3]), ks(s[2]), ks(c[2]))
        nc.scalar.activation(sqq[2], qs(s[2]), ACTF.Square)
        nc.scalar.activation(ct3k, ks(s[2]), ACTF.Square, scale=math.sqrt(32.0))
        # switch ACT tables to the exp set — Square works in both sets, and
        # warm2 READS sqq[1] so the 1.3us load lands in ACT's idle window
        # between sq1q and sq2q (the scheduler orders by data readiness)
        warm2 = inp.tile([128, 128], FP16, tag="warm2", name="warm2")
        nc.scalar.activation(warm2, sqq[1][:, 0:128], ACTF.Exp)
        nc.vector.tensor_mul(qs(s[3]), qs(s[2]), qs(c[2]))
        scale_full(2)
        emit_half(2, 0, 0)
        emit_half(2, 1, 0)
        emit_half(2, 0, 1)
        emit_half(2, 1, 1)
        scale_half(3, 0)
        emit_half(3, 0, 0)
        emit_half(3, 1, 0)
        # Cqb_3 directly from sq2q: col*(1 - 32*sq) in one tensor_scalar per
        # hc — skips materializing c3q and removes a dependency hop on the
        # path that gates the final matmuls
        for hc in range(2):
            q2 = slice(hc * NQ, (hc + 1) * NQ)
            nc.vector.tensor_scalar(SCb[3][:, 1, q2], sqq[2][:, q2],
                                    colAP(hc, 5), colAP(hc, 4),
                                    ALU.mult, ALU.add)
        emit_half(3, 0, 1, last=True)
        emit_half(3, 1, 1, last=True)

        # ---- softmax (no per-row max: constant bound expC) + AV per q-tile.
        # pt is one double-width PSUM tile; the q-tiles use disjoint halves
        # so their transposes don't serialize on each other.
        pt = pt_tile
        for qt in range(2):
            scp = sc_ps[qt]
            p_sb = sm.tile([128, KP], BF16, tag=f"p{qt}", name=f"p{qt}")
            ssum = sm.tile([128, 1], FP32, tag=f"ss{qt}", name=f"ss{qt}")
            nc.scalar.activation(p_sb, scp, ACTF.Exp, bias=nexpc,
                                 accum_out=ssum)
            rs = sm.tile([128, 1], FP32, tag=f"rs{qt}", name=f"rs{qt}")
            nc.vector.reciprocal(rs, ssum)

            ptq = pt[:, qt * NK * 128: (qt + 1) * NK * 128]
            for kc in range(NK):
                nc.tensor.transpose(ptq[:, 128 * kc: 128 * (kc + 1)],
                                    p_sb[:, 128 * kc: 128 * (kc + 1)], ident)
            pts = sm.tile([128, NK * 128], BF16, tag=f"pts{qt}", name=f"pts{qt}")
            nc.vector.tensor_copy(pts, ptq)
            av = ps_pr.tile([128, V], FP32, tag=f"av{qt}", name=f"av{qt}")
            for kc in range(NK):
                nc.tensor.matmul(av, pts[:, 128 * kc: 128 * (kc + 1)], v_sb[kc],
                                 start=(kc == 0), stop=(kc == NK - 1))
            o = sm.tile([128, V], FP16, tag=f"o{qt}", name=f"o{qt}")
            nc.scalar.activation(o, av, ACTF.Copy, scale=rs)
            nc.sync.dma_start(out=out_d[128 * qt: 128 * (qt + 1), :], in_=o)

    nc.compile()
    return nc


class TileCtx:
    """TileContext + ExitStack in one `with`."""

    def __init__(self, nc):
        self.nc = nc

    def __enter__(self):
        self.ctx = ExitStack()
        self.tc = self.ctx.enter_context(tile.TileContext(self.nc))
        return self.tc, self.ctx

    def __exit__(self, *exc):
        return self.ctx.__exit__(*exc)


def prepare(inputs):
    """Host prep: shards, fit, per-core input maps."""
    queries = np.ascontiguousarray(np.asarray(inputs["queries"], np.float32))
    keys = np.ascontiguousarray(np.asarray(inputs["keys"], np.float32))
    values = np.ascontiguousarray(np.asarray(inputs["values"], np.float32))
    vls = np.asarray(inputs["valid_lens"]).astype(np.int64)
    Wq = np.asarray(inputs["W_q"], np.float32)
    Wk = np.asarray(inputs["W_k"], np.float32)
    wv = np.asarray(inputs["w_v"], np.float32)

    # device projections run on fp16-rounded inputs; match that for ranges
    q16 = queries.astype(np.float16).astype(np.float32)
    k16 = keys.astype(np.float16).astype(np.float32)
    Wq16 = Wq.astype(np.float16).astype(np.float32)
    Wk16 = Wk.astype(np.float16).astype(np.float32)
    qp = [(Wq16 @ q16[b].T).astype(np.float32) for b in range(B)]   # [h, q]
    kp = [(Wk16 @ k16[b].T).astype(np.float32) for b in range(B)]   # [h, k]
    w0, bco = fit_series(qp, kp, vls)
    KP = 128 * max(1, int(math.ceil(vls.max() / 128.0)))
    expC = float(np.abs(wv).sum() * 1.05)    # score upper bound for exp bias

    # scale columns: per hc, [col_0, col_1, col_2, colS_3, colC_3, colX_3]
    NCOL = 2 * (NLEV + 2)
    cols = np.zeros((128, NCOL), np.float32)
    for hc in range(2):
        wvh = wv[128 * hc: 128 * (hc + 1)]
        base = hc * (NLEV + 2)
        for l in range(3):
            cols[:, base + l] = wvh * bco[l] * (2.0 ** l)
        cols[:, base + 3] = -wvh * bco[3] * 8.0     # Sqb_3 (pairs with ct3k)
        cols[:, base + 4] = wvh * bco[3] * 8.0      # Cqb_3 bias term
        cols[:, base + 5] = -32.0 * wvh * bco[3] * 8.0  # Cqb_3 * sq2q term

    OFF, PX, _cuts = pack_layout(KP)
    NK = KP // 128
    in_maps = []
    for core in range(NCORES):
        b, qlo = core // 2, (core % 2) * NQ
        n = int(vls[b])
        pk = np.zeros((128, PX), np.float16)
        qTm = queries[b, qlo: qlo + NQ].T.astype(np.float16)        # (D, NQ)
        kTm = np.zeros((D, KP), np.float16)
        kTm[:, :n] = keys[b, :n].T.astype(np.float16)
        for i in range(2):
            pk[:, OFF[f"qT{i}"]: OFF[f"qT{i}"] + NQ] = qTm[128 * i: 128 * (i + 1)]
            pk[:, OFF[f"kT{i}"]: OFF[f"kT{i}"] + KP] = kTm[128 * i: 128 * (i + 1)]
            pk[:, OFF[f"wq{i}"]: OFF[f"wq{i}"] + H] = Wq.T[128 * i: 128 * (i + 1)].astype(np.float16)
            pk[:, OFF[f"wk{i}"]: OFF[f"wk{i}"] + H] = Wk.T[128 * i: 128 * (i + 1)].astype(np.float16)
        vm = np.zeros((KP, V), np.float16)
        vm[:n] = values[b, :n].astype(np.float16)
        for i in range(NK):
            pk[:, OFF[f"v{i}"]: OFF[f"v{i}"] + V] = vm[128 * i: 128 * (i + 1)]
        # identity with bf16(1.0)=0x3F80 bit patterns, carried in the fp16 pack
        pk[:, OFF["ident"]: OFF["ident"] + 128] = \
            (np.eye(128) * 0x3F80).astype(np.uint16).view(np.float16)
        mov = np.zeros((1, KP + 128), np.float16)
        mov[0, :KP] = np.where(np.arange(KP) < n, 0.0, NEGM).astype(np.float16)
        mov[0, KP:] = 1.0
        in_maps.append({"pack": pk, "mo": mov, "cols": cols})
    return w0, bco, KP, expC, in_maps


def kernel(**inputs):
    w0, bco, KP, expC, in_maps = prepare(inputs)
    nc = build_nc(w0, bco, KP, expC)
    res = run_bass_kernel_spmd(nc, in_maps, core_ids=list(range(NCORES)))
    out = np.zeros((B, Q, V), np.float32)
    for core in range(NCORES):
        b, qlo = core // 2, (core % 2) * NQ
        out[b, qlo: qlo + NQ] = np.asarray(res.results[core]["out"],
                                           np.float32)
    return out
